# revision 11
# baseline (speedup 1.0000x reference)
"""Trainium2 Bass kernel for nn_AttnDecoder_87230785782556.

Multi-head attention decoder layer: out = softmax((xq Wq)(xk Wk)^T * s) (xv Wv) Wo
Sharding: 8 cores = 2 batches x 4 head-groups (4 heads each, tensor-parallel
column split of Wq/Wk/Wv, row split of Wo; partial outputs summed on host).

All matmuls run in float32r (full PE rate, ~1.5e-4 rounding). Scores are
computed transposed [kv, q] so exp output feeds attn@V as the moving operand;
an extra all-ones column in the V weights makes the same matmul emit the
softmax denominator. Normalization uses an indicator-matmul to broadcast
1/den across partitions.

Overlap structure: chunk-0 attention is fused into the K/V projection stream
(scores for kv-range r start right after khT/vh of range r), attn@V trails
scores/exp by one group, and normalization+output-projection trail attention
by one q-chunk. This keeps the Scalar engine (the exp bottleneck) fed from
~30us earlier and hides the den/recip chain.
"""
import math
import numpy as np

from concourse import bacc, mybir, tile
from concourse.bass_utils import run_bass_kernel_spmd

# Problem constants (hardcoded per contract)
B = 2
SEQ = 2048
E = 1024
NUM_HEADS = 16
HD = 64
QK_SCALE = 0.125
N_CORES = 8
HPC = 4            # heads per core
P = 128
NQ = 512           # q chunk (matmul moving free dim)

F32 = mybir.dt.float32
F32R = mybir.dt.float32r


def build_program(seq=SEQ, repeat=1):
    """Build the per-core SPMD program. Identical on all 8 cores."""
    nc = bacc.Bacc("TRN2", target_bir_lowering=False, debug=False,
                   num_devices=N_CORES)

    n_qc = seq // NQ            # q chunks
    n_kv = seq // P             # kv tiles of 128
    n_kt = E // P               # embedding contraction tiles
    FPC = HPC * HD              # features per core = 256
    n_m = FPC // P              # feature pair-tiles = 2
    n_g = n_kv // 2             # attn groups (2 kv tiles each)

    xtq = nc.dram_tensor("xtq", [E, seq], F32R, kind="ExternalInput")
    xtk = nc.dram_tensor("xtk", [E, seq], F32R, kind="ExternalInput")
    xtv = nc.dram_tensor("xtv", [E, seq], F32R, kind="ExternalInput")
    wq = nc.dram_tensor("wq", [E, FPC], F32R, kind="ExternalInput")
    wk = nc.dram_tensor("wk", [E, FPC], F32R, kind="ExternalInput")
    wv = nc.dram_tensor("wv", [E, HPC * (HD + 1)], F32R, kind="ExternalInput")
    wo = nc.dram_tensor("wo", [FPC, E], F32R, kind="ExternalInput")
    ind = nc.dram_tensor("ind", [HPC, n_m, P], F32R, kind="ExternalInput")
    out = nc.dram_tensor("out", [seq, E], F32, kind="ExternalOutput")

    with tile.TileContext(nc) as tc, nc.allow_low_precision("f32r pipeline"):
        import contextlib
        ctx = contextlib.ExitStack()
        with ctx:
            consts = ctx.enter_context(tc.tile_pool(name="consts", bufs=1))
            bigs = ctx.enter_context(tc.tile_pool(name="bigs", bufs=1))
            vhp = ctx.enter_context(tc.tile_pool(name="vhp", bufs=n_kv))
            xs = ctx.enter_context(tc.tile_pool(name="xs", bufs=6))
            expp = ctx.enter_context(tc.tile_pool(name="expp", bufs=10))
            stkp = ctx.enter_context(tc.tile_pool(name="stkp", bufs=4))
            denp = ctx.enter_context(tc.tile_pool(name="denp", bufs=2))
            finp = ctx.enter_context(tc.tile_pool(name="finp", bufs=3))
            ps = ctx.enter_context(tc.tile_pool(name="ps", bufs=4, space="PSUM"))

            # ---- resident constants ----
            wq_t = consts.tile([P, n_kt, FPC], F32R, name="wq_t", tag="wq")
            wk_t = consts.tile([P, n_kt, FPC], F32R, name="wk_t", tag="wk")
            wv_t = consts.tile([P, n_kt, HPC * (HD + 1)], F32R, name="wv_t", tag="wv")
            wo_t = consts.tile([P, n_m, E], F32R, name="wo_t", tag="wo")
            ind_t = consts.tile([HPC, n_m, P], F32R, name="ind_t", tag="ind")
            nc.sync.dma_start(out=wq_t, in_=wq.ap().rearrange("(t p) m -> p t m", p=P))
            nc.sync.dma_start(out=wk_t, in_=wk.ap().rearrange("(t p) m -> p t m", p=P))
            nc.sync.dma_start(out=wv_t, in_=wv.ap().rearrange("(t p) m -> p t m", p=P))
            nc.sync.dma_start(out=wo_t, in_=wo.ap().rearrange("(t p) m -> p t m", p=P))
            nc.sync.dma_start(out=ind_t, in_=ind.ap())

            for rep in range(repeat):
                khT = [bigs.tile([P, seq], F32R, name=f"khT{m}", tag=f"khT{m}")
                       for m in range(n_m)]
                qhT = [bigs.tile([P, seq], F32R, name=f"qhT{m}", tag=f"qhT{m}")
                       for m in range(n_m)]
                vh_tiles = [vhp.tile([P, HPC, HD + 1], F32R, name=f"vh{i}", tag="vh")
                            for i in range(n_kv)]

                # ---- K projection for one 512-wide kv range ----
                def emit_kproj(nq):
                    ps_t = ps.tile([P, 2, NQ], F32, name="ps_t", tag="ps")
                    for kt in range(n_kt):
                        x_t = xs.tile([P, NQ], F32R, name="xk_t", tag="x")
                        nc.sync.dma_start(
                            out=x_t,
                            in_=xtk.ap()[P * kt:P * (kt + 1), NQ * nq:NQ * (nq + 1)])
                        for m in range(n_m):
                            nc.tensor.matmul(
                                ps_t[:, m, :],
                                wk_t[:, kt, P * m:P * (m + 1)],
                                x_t,
                                start=(kt == 0), stop=(kt == n_kt - 1))
                    for m in range(n_m):
                        nc.vector.tensor_copy(
                            khT[m][:, NQ * nq:NQ * (nq + 1)], ps_t[:, m, :])

                # ---- V projection for one 512-wide kv range (4 vh tiles) ----
                def emit_vproj(mb):
                    ps_v = [ps.tile([P, 2, NQ], F32, name="ps_t", tag="ps")
                            for _ in range(2)]
                    for kt in range(n_kt):
                        xv_t = xs.tile([P, NQ], F32R, name="xv_t", tag="x")
                        nc.sync.dma_start(
                            out=xv_t,
                            in_=xtv.ap()[P * kt:P * (kt + 1), NQ * mb:NQ * (mb + 1)])
                        for sub in range(4):
                            nc.tensor.matmul(
                                ps_v[sub // 2][:, sub % 2, 0:HPC * (HD + 1)],
                                xv_t[:, P * sub:P * (sub + 1)],
                                wv_t[:, kt, :],
                                start=(kt == 0), stop=(kt == n_kt - 1))
                    for sub in range(4):
                        mk = 4 * mb + sub
                        nc.vector.tensor_copy(
                            vh_tiles[mk],
                            ps_v[sub // 2][:, sub % 2, 0:HPC * (HD + 1)].rearrange(
                                "p (h c) -> p h c", h=HPC))
                        # ones slots (zero in wv): even heads col 64, odd heads col 0
                        nc.vector.tensor_scalar_add(
                            vh_tiles[mk][:, 0::2, HD], vh_tiles[mk][:, 0::2, HD], 1.0)
                        nc.vector.tensor_scalar_add(
                            vh_tiles[mk][:, 1::2, 0], vh_tiles[mk][:, 1::2, 0], 1.0)

                # ---- Q projection for one q chunk ----
                def emit_qproj(qc):
                    ps_q = ps.tile([P, 2, NQ], F32, name="ps_t", tag="ps")
                    for kt in range(n_kt):
                        xq_t = xs.tile([P, NQ], F32R, name="xq_t", tag="x")
                        nc.sync.dma_start(
                            out=xq_t,
                            in_=xtq.ap()[P * kt:P * (kt + 1), NQ * qc:NQ * (qc + 1)])
                        for m in range(n_m):
                            nc.tensor.matmul(
                                ps_q[:, m, :],
                                wq_t[:, kt, P * m:P * (m + 1)],
                                xq_t,
                                start=(kt == 0), stop=(kt == n_kt - 1))
                    for m in range(n_m):
                        nc.vector.tensor_copy(
                            qhT[m][:, NQ * qc:NQ * (qc + 1)], ps_q[:, m, :])

                # ---- attention helpers (incremental, per 2-kvtile group) ----
                def start_pair(qc, pair):
                    return {"qc": qc, "pair": pair, "exp": [],
                            "att": ps.tile([P, 2, NQ], F32, name="ps_t", tag="ps")}

                def attnv(st, g):
                    for s in range(2):
                        kv = 2 * g + s
                        for h01 in range(2):
                            nc.tensor.matmul(
                                st["att"][0:HD + 1, h01, :],
                                vh_tiles[kv][:, 2 * st["pair"] + h01, :],
                                st["exp"][g][h01][:, s, :],
                                start=(kv == 0), stop=(kv == n_kv - 1))

                def pair_groups(st, g0, g1):
                    # scores+exp for groups [g0, g1); attn@V trails by one group
                    qc, pair = st["qc"], st["pair"]
                    for g in range(g0, g1):
                        sc = []
                        for h01 in range(2):
                            sc_t = ps.tile([P, 2, NQ], F32, name="ps_t", tag="ps")
                            for s in range(2):
                                kv = 2 * g + s
                                nc.tensor.matmul(
                                    sc_t[:, s, :],
                                    khT[pair][64 * h01:64 * (h01 + 1),
                                              P * kv:P * (kv + 1)],
                                    qhT[pair][64 * h01:64 * (h01 + 1),
                                              NQ * qc:NQ * (qc + 1)],
                                    start=True, stop=True,
                                    tile_position=(64 * h01, 0))
                            sc.append(sc_t)
                        pair_exp = []
                        for h01 in range(2):
                            e_t = expp.tile([P, 2, NQ], F32R, name="exp_t", tag="exp")
                            nc.scalar.activation(
                                e_t, sc[h01], mybir.ActivationFunctionType.Exp)
                            pair_exp.append(e_t)
                        st["exp"].append(pair_exp)
                        if g > 0:
                            attnv(st, g - 1)

                def finish_pair(st, den_t):
                    attnv(st, n_g - 1)
                    att_ps, pair = st["att"], st["pair"]
                    # stack the two heads' outputs [feat, q] into one tile.
                    # PSUM is not DMA-accessible: stage through SBUF, then
                    # SBUF->SBUF DMAs do the cross-partition moves.
                    stk_t = stkp.tile([P, NQ], F32R, name="stk_t", tag="stk")
                    nc.vector.tensor_copy(stk_t[0:HD, :], att_ps[0:HD, 0, :])
                    sodd_t = stkp.tile([P, NQ], F32R, name="sodd_t", tag="sodd",
                                       bufs=2)
                    nc.vector.tensor_copy(sodd_t[0:HD + 1, :],
                                          att_ps[0:HD + 1, 1, :])
                    dstg_t = stkp.tile([P, NQ], F32R, name="dstg_t", tag="dstg",
                                       bufs=2)
                    nc.vector.tensor_copy(dstg_t[HD:HD + 1, :],
                                          att_ps[HD:HD + 1, 0, :])
                    nc.sync.dma_start(out=stk_t[HD:P, :], in_=sodd_t[1:HD + 1, :])
                    # denominators: even head psum row 64 slot 0, odd row 0 slot 1
                    nc.sync.dma_start(out=den_t[2 * pair:2 * pair + 1, :],
                                      in_=dstg_t[HD:HD + 1, :])
                    nc.sync.dma_start(out=den_t[2 * pair + 1:2 * pair + 2, :],
                                      in_=sodd_t[0:1, :])
                    return stk_t

                def emit_norm_outproj(stk_tiles, den_t, qc):
                    # normalize: bcast 1/den across partitions via indicator matmul
                    rcp_t = denp.tile([HPC, NQ], F32R, name="rcp_t", tag="rcp")
                    nc.vector.reciprocal(rcp_t, den_t)
                    for pair in range(n_m):
                        bc_ps = ps.tile([P, 2, NQ], F32, name="ps_t", tag="ps")
                        nc.tensor.matmul(bc_ps[:, 0, :], ind_t[:, pair, :], rcp_t,
                                         start=True, stop=True)
                        nc.vector.tensor_mul(stk_tiles[pair], stk_tiles[pair],
                                             bc_ps[:, 0, :])
                    # output projection: out[q, E] = sum_pair stk[pair].T @ wo[pair]
                    for qs in range(NQ // P):
                        op_ps = ps.tile([P, 2, NQ], F32, name="ps_t", tag="ps")
                        for nch in range(E // NQ):
                            for pair in range(n_m):
                                nc.tensor.matmul(
                                    op_ps[:, nch, :],
                                    stk_tiles[pair][:, P * qs:P * (qs + 1)],
                                    wo_t[:, pair, NQ * nch:NQ * (nch + 1)],
                                    start=(pair == 0), stop=(pair == n_m - 1))
                        fin_t = finp.tile([P, 2, NQ], F32, name="fin_t", tag="fin")
                        for nch in range(E // NQ):
                            nc.vector.tensor_copy(fin_t[:, nch, :], op_ps[:, nch, :])
                        r0 = NQ * qc + P * qs
                        nc.sync.dma_start(
                            out=out.ap()[r0:r0 + P, :],
                            in_=fin_t.rearrange("p a b -> p (a b)"))

                # ---- main flow ----
                pending = None
                for qc in range(n_qc):
                    emit_qproj(qc)
                    den_t = denp.tile([HPC, NQ], F32R, name="den_t", tag="den")
                    if qc == 0:
                        # fuse chunk-0 attention into the K/V projection stream:
                        # kv-range r's scores run right after khT/vh of range r,
                        # so the Scalar engine starts exp ~30us earlier.
                        sts = [start_pair(0, pair) for pair in range(n_m)]
                        for nq in range(n_qc):
                            emit_kproj(nq)
                            emit_vproj(nq)
                            for st in sts:
                                pair_groups(st, 2 * nq, 2 * nq + 2)
                        stk_tiles = [finish_pair(st, den_t) for st in sts]
                    else:
                        stk_tiles = []
                        for pair in range(n_m):
                            st = start_pair(qc, pair)
                            pair_groups(st, 0, n_g)
                            stk_tiles.append(finish_pair(st, den_t))
                    # norm + output projection trail attention by one chunk so the
                    # den DMA/recip chain overlaps the next chunk's attention.
                    if pending is not None:
                        emit_norm_outproj(*pending)
                    pending = (stk_tiles, den_t, qc)
                if pending is not None:
                    emit_norm_outproj(*pending)
                    pending = None
    nc.finalize()
    return nc


_PROG_CACHE = {}


def _get_program(seq=SEQ, repeat=1):
    key = (seq, repeat)
    if key not in _PROG_CACHE:
        _PROG_CACHE[key] = build_program(seq, repeat)
    return _PROG_CACHE[key]


def shard_inputs(q, k, v, Wq, Wk, Wv, Wo, seq=SEQ):
    """Build the 8 per-core input maps (host-side layout prep)."""
    scale = np.float32(QK_SCALE / math.sqrt(B))
    in_maps = []
    for c in range(N_CORES):
        b = c // 4
        hg = c % 4
        heads = [4 * hg + j for j in range(HPC)]
        wq_s = np.concatenate([Wq[:, h::NUM_HEADS] for h in heads], axis=1) * scale
        wk_s = np.concatenate([Wk[:, h::NUM_HEADS] for h in heads], axis=1)
        wv_s = np.zeros((E, HPC, HD + 1), dtype=np.float32)
        for j, h in enumerate(heads):
            if j % 2 == 0:
                wv_s[:, j, 0:HD] = Wv[:, h::NUM_HEADS]
            else:
                wv_s[:, j, 1:HD + 1] = Wv[:, h::NUM_HEADS]
        wo_s = np.concatenate([Wo[h::NUM_HEADS, :] for h in heads], axis=0)
        ind = np.zeros((HPC, HPC // 2, P), dtype=np.float32)
        for kk in range(HPC):
            for pair in range(HPC // 2):
                for m in range(P):
                    if kk == 2 * pair + m // HD:
                        ind[kk, pair, m] = 1.0
        in_maps.append({
            "xtq": np.ascontiguousarray(q[b][:seq].T),
            "xtk": np.ascontiguousarray(k[b][:seq].T),
            "xtv": np.ascontiguousarray(v[b][:seq].T),
            "wq": np.ascontiguousarray(wq_s),
            "wk": np.ascontiguousarray(wk_s),
            "wv": np.ascontiguousarray(wv_s.reshape(E, HPC * (HD + 1))),
            "wo": np.ascontiguousarray(wo_s),
            "ind": ind,
        })
    return in_maps


def unshard(results, seq=SEQ):
    out = np.zeros((B, seq, E), dtype=np.float32)
    for c in range(N_CORES):
        out[c // 4] += results[c]["out"]
    return out


def kernel(q, k, v, Wq, Wk, Wv, Wo):
    q = np.asarray(q, dtype=np.float32)
    k = np.asarray(k, dtype=np.float32)
    v = np.asarray(v, dtype=np.float32)
    Wq = np.asarray(Wq, dtype=np.float32)
    Wk = np.asarray(Wk, dtype=np.float32)
    Wv = np.asarray(Wv, dtype=np.float32)
    Wo = np.asarray(Wo, dtype=np.float32)
    nc = _get_program()
    in_maps = shard_inputs(q, k, v, Wq, Wk, Wv, Wo)
    res = run_bass_kernel_spmd(nc, in_maps, list(range(N_CORES)))
    return unshard(res.results)
